# revision 65
# baseline (speedup 1.0000x reference)
"""AttentionBlock (GroupNorm32 + qkv 1x1 + channel-attention + proj + residual)
for Trainium2, SPMD over 8 NeuronCores (data-parallel over batch B=8).

Self-contained: hardcodes shapes B=8, C=1024, L=4096, H=16, groups=32.
kernel(**inputs) takes the FULL numpy inputs and returns the FULL output.

Math per batch b (one core), restructured around the channel-Gram matrix
(score contracts over L, so score = Wq' G Wk'^T with G = x x^T [C, C]):

  xbf   = bf16(x)                  (kept resident, raw)
  xT    = transpose(xbf)           (DMA-transpose, bf16)
  G     = xbf xbf^T                (PE, bf16, upper triangle + mirror, fp16)
  stats : sumx = xT @ 1 (PE), E[x^2] = diag(G) -> groupnorm affine s, b
  wq2   = wqT_p * s (fp16)         (affine folded into score weights)
  M2    = G @ wk2                  (fp16)
  score = wq2^T M2 + rank-1 bias corrections (beta/u outer products)
  w     = softmax(score)           per head (f32)
  A_h   = w_h^T Wp_h^T             (bf16) -> M^T = sum_h Wv_h^T A_h + I
  out   = (diag(s) M'^T)^T xbf + (pb + c0 + M' b)   (single C x C x L matmul;
          residual, proj, v, ctx, affine and all biases folded into M'/consts)
"""

import os
import sys

try:
    import concourse.bass  # noqa: F401
except ImportError:  # pragma: no cover
    sys.path.insert(0, "/opt/trn_rl_repo")

import numpy as np
import ml_dtypes

import concourse.bass as bass
import concourse.bacc as bacc
import concourse.tile as tile
from concourse import mybir
from concourse.bass_utils import run_bass_kernel_spmd

B, C, L, H = 8, 1024, 4096, 16
G = 32          # groupnorm groups
CH = C // H     # 64 channels per head
EPS = 1e-5
CT = C // 128   # 8 channel chunks
NLT = L // 128  # 32 l-tiles
F32 = mybir.dt.float32
F32R = mybir.dt.float32r
BF16 = mybir.dt.bfloat16
F16 = mybir.dt.float16
BFNP = ml_dtypes.bfloat16

Alu = mybir.AluOpType
Act = mybir.ActivationFunctionType
DEBUG = bool(int(os.environ.get("ATT_DEBUG", "0")))

# upper-triangle Gram tiles for pass 1: rows 0..6, cols i..6 (28 tiles);
# row 7 (full, 8 tiles) is pass 2. slot k -> psum bank k//4, col k%4.
P1_PAIRS = [(i, j) for i in range(7) for j in range(i, 7)]
P1_SLOT = {p: k for k, p in enumerate(P1_PAIRS)}


def _build():
    nc = bacc.Bacc("TRN2", target_bir_lowering=False, debug=False, num_devices=8)

    x = nc.declare_dram_parameter("x", [C, L], F32, isOutput=False)
    xtf = nc.declare_dram_parameter("xtf", [L, C], F32, isOutput=False)
    wqt = nc.declare_dram_parameter("wqt", [C, C], F16, isOutput=False)
    wkt = nc.declare_dram_parameter("wkt", [C, C], F16, isOutput=False)
    wpt = nc.declare_dram_parameter("wpt", [C, C], BF16, isOutput=False)
    wvn = nc.declare_dram_parameter("wvn", [C, C], BF16, isOutput=False)
    bqp = nc.declare_dram_parameter("bqp", [1, C], F16, isOutput=False)
    bqpL = nc.declare_dram_parameter("bqpL", [1, C], F16, isOutput=False)
    bkp = nc.declare_dram_parameter("bkp", [1, C], F16, isOutput=False)
    bvc = nc.declare_dram_parameter("bvc", [128, CT], BF16, isOutput=False)
    pbc = nc.declare_dram_parameter("pbc", [128, CT], F32, isOutput=False)
    gnw = nc.declare_dram_parameter("gnw", [128, CT], F32, isOutput=False)
    gnb = nc.declare_dram_parameter("gnb", [128, CT], F32, isOutput=False)
    gsel = nc.declare_dram_parameter("gsel", [128, 4], F32, isOutput=False)
    gbr = nc.declare_dram_parameter("gbr", [4, 128], F32, isOutput=False)
    idb = nc.declare_dram_parameter("idb", [128, 128], BF16, isOutput=False)
    idh = nc.declare_dram_parameter("idh", [128, 128], F16, isOutput=False)
    ones = nc.declare_dram_parameter("ones", [128, 1], BF16, isOutput=False)
    out = nc.declare_dram_parameter("out", [C, L], F32, isOutput=True)
    dbg = None
    if DEBUG:
        dbg = {
            "dbg_g": nc.declare_dram_parameter("dbg_g", [C, C], F16, isOutput=True),
            "dbg_w": nc.declare_dram_parameter("dbg_w", [128, 512], BF16, isOutput=True),
            "dbg_mt": nc.declare_dram_parameter("dbg_mt", [C, C], BF16, isOutput=True),
            "dbg_pbc0": nc.declare_dram_parameter("dbg_pbc0", [128, CT], F32, isOutput=True),
            "dbg_rows": nc.declare_dram_parameter("dbg_rows", [5, C], F16, isOutput=True),
            "dbg_sc": nc.declare_dram_parameter("dbg_sc", [128, 2 * CT], F32, isOutput=True),
        }

    with tile.TileContext(nc) as tc:
        _body(nc, tc, x, xtf, wqt, wkt, wpt, wvn, bqp, bqpL, bkp, bvc, pbc,
              gnw, gnb, gsel, gbr, idb, idh, ones, out, dbg)
    nc.compile()
    return nc


def _body(nc, tc, x, xtf, wqt, wkt, wpt, wvn, bqp, bqpL, bkp, bvc, pbc,
          gnw, gnb, gsel, gbr, idb, idh, ones, out, dbg=None):
    from contextlib import ExitStack

    with ExitStack() as ctx:
        singles = ctx.enter_context(tc.tile_pool(name="singles", bufs=1))

        def ecopy(k, out_, in_):
            # SBUF-to-SBUF copy: any of the three elementwise engines
            e = k % 3
            if e == 1:
                nc.scalar.activation(out=out_, in_=in_, func=Act.Copy)
            elif e == 0:
                nc.vector.tensor_copy(out=out_, in_=in_)
            else:
                nc.gpsimd.tensor_copy(out=out_, in_=in_)

        def pcopy(k, out_, in_):
            # PSUM-source copy: gpsimd (Pool) cannot access PSUM
            if k % 2 == 0:
                nc.vector.tensor_copy(out=out_, in_=in_)
            else:
                nc.scalar.activation(out=out_, in_=in_, func=Act.Copy)

        def load1(name, dram, shape, dtype=F32):
            t = singles.tile(shape, dtype, name=name)
            nc.sync.dma_start(out=t, in_=dram[:, :])
            return t

        gsel_sb = load1("gsel", gsel, [128, 4])
        gbr_sb = load1("gbr", gbr, [4, 128])
        gnw_sb = load1("gnw", gnw, [128, CT])
        gnb_sb = load1("gnb", gnb, [128, CT])
        idb_sb = load1("idb", idb, [128, 128], BF16)
        idh_sb = load1("idh", idh, [128, 128], F16)
        ones_sb = load1("ones", ones, [128, 1], BF16)
        bvc_sb = load1("bvc", bvc, [128, CT], BF16)
        pbc_sb = load1("pbc", pbc, [128, CT])
        eps_sb = singles.tile([128, 1], F32, name="eps")
        nc.vector.memset(eps_sb, EPS)

        scale_sb = singles.tile([128, CT], F32, name="scale")
        bias_sb = singles.tile([128, CT], F32, name="biasc")
        bcol16 = singles.tile([128, CT], F16, name="bcol16")
        bcolb = singles.tile([128, CT], BF16, name="bcolb")
        sumx_sb = singles.tile([128, CT], F32, name="sumx")
        sumx16 = singles.tile([128, CT], F16, name="sumx16")
        ex2_sb = singles.tile([128, CT], F32, name="ex2")
        pbc0_sb = singles.tile([128, CT], F32, name="pbc0")
        negmax = singles.tile([128, CT], F32, name="negmax")
        sumexp = singles.tile([128, CT], F32, name="sumexp")
        rs = singles.tile([128, CT], F32, name="rsum")

        # ---- resident big tiles (no releases: everything fits) ----------
        pxbf = ctx.enter_context(tc.tile_pool(name="pxbf", bufs=1))
        xbf = pxbf.tile([128, CT, L], BF16, name="xbf")   # raw x, bf16
        pqk = ctx.enter_context(tc.tile_pool(name="pqk", bufs=1))
        wqt_sb = pqk.tile([128, CT, C], F16, name="wqt")
        wkt_sb = pqk.tile([128, CT, C], F16, name="wkt")
        pg = ctx.enter_context(tc.tile_pool(name="pg", bufs=1))
        g_sb = pg.tile([128, CT, C], F16, name="gsb")     # full Gram, fp16
        prow = ctx.enter_context(tc.tile_pool(name="prow", bufs=1))
        betaq_r = prow.tile([1, C], F16, name="betaqr")
        betak_r = prow.tile([1, C], F16, name="betakr")
        lbetaq_r = prow.tile([1, C], F16, name="lbetaqr")
        uq_r = prow.tile([1, C], F16, name="uqr")
        uk_r = prow.tile([1, C], F16, name="ukr")
        w_sb = prow.tile([128, 512], BF16, name="wsb")

        # =========== phase 1: stream x + host-transposed x, cast, Gram ===
        with tc.tile_pool(name="pxT", bufs=1) as pxT, \
             tc.tile_pool(name="stage", bufs=3) as pstg, \
             tc.tile_pool(name="gps", bufs=1, space="PSUM") as gps, \
             tc.tile_pool(name="g2ps", bufs=1, space="PSUM") as g2ps:
            xT = pxT.tile([128, NLT, C], BF16, name="xT")
            gbank = [gps.tile([128, 512], F32, name=f"gb{b}") for b in range(7)]
            g7 = g2ps.tile([128, 512], F32, name="g7")

            for lt in range(NLT):        # x streamed per l-tile [C, 128]
                xts = pstg.tile([128, CT, 128], F32, name="xstg")
                xtv = xts.rearrange("p a b -> p (a b)")
                nc.sync.dma_start(out=xtv, in_=xtf[lt * 128:(lt + 1) * 128, :])
                xs = pstg.tile([128, CT, 128], F32, name="xstg")
                for ct in range(CT):
                    eng = nc.scalar if ct % 2 == 0 else nc.sync
                    eng.dma_start(
                        out=xs[:, ct, :],
                        in_=x[ct * 128:(ct + 1) * 128, lt * 128:(lt + 1) * 128])
                for half in range(2):
                    ecopy(lt + half, xT[:, lt, half * 512:(half + 1) * 512],
                          xtv[:, half * 512:(half + 1) * 512])
                for ct in range(CT):
                    ecopy(ct + lt, xbf[:, ct, lt * 128:(lt + 1) * 128],
                          xs[:, ct, :])
                # Gram pass-1 matmuls for this l-tile (upper, cols<=6).
                # start=True only on the chronologically first touch of each
                # bank: a later start re-marks the whole bank pending-zero
                # and wipes sibling chains' accumulation.
                started_banks = set()
                for i in range(7):
                    j = i
                    while j < 7:
                        s0 = P1_SLOT[(i, j)]
                        w = 1
                        while (j + w < 7 and s0 % 4 + w < 4
                               and P1_SLOT[(i, j + w)] == s0 + w):
                            w += 1
                        st = lt == 0 and s0 // 4 not in started_banks
                        started_banks.add(s0 // 4)
                        nc.tensor.matmul(
                            out=gbank[s0 // 4][:, (s0 % 4) * 128:
                                               (s0 % 4) * 128 + w * 128],
                            lhsT=xT[:, lt, i * 128:(i + 1) * 128],
                            rhs=xT[:, lt, j * 128:(j + w) * 128],
                            start=st, stop=(lt == NLT - 1),
                            skip_group_check=True)
                        j += w
                # row 7, first half (cols 0..511)
                nc.tensor.matmul(
                    out=g7, lhsT=xT[:, lt, 896:1024],
                    rhs=xT[:, lt, 0:512],
                    start=(lt == 0), stop=(lt == NLT - 1),
                    skip_group_check=True)

            # score weights: load after the x stream so x owns the queues
            for ct in range(CT):
                nc.scalar.dma_start(out=wqt_sb[:, ct, :],
                                    in_=wqt[ct * 128:(ct + 1) * 128, :])
                nc.sync.dma_start(out=wkt_sb[:, ct, :],
                                  in_=wkt[ct * 128:(ct + 1) * 128, :])

            # drain pass-1 banks to SBUF (fp16), row-major tile placement
            for (i, j), k in P1_SLOT.items():
                pcopy(k, g_sb[:, i, j * 128:(j + 1) * 128],
                      gbank[k // 4][:, (k % 4) * 128:(k % 4) * 128 + 128])
            nc.vector.tensor_copy(out=g_sb[:, 7, 0:512], in_=g7)

            # row 7 second half (cols 512..1023) reuses the g7 bank
            g7b = g2ps.tile([128, 512], F32, name="g7")
            for lt in range(NLT):
                nc.tensor.matmul(
                    out=g7b, lhsT=xT[:, lt, 896:1024], rhs=xT[:, lt, 512:1024],
                    start=(lt == 0), stop=(lt == NLT - 1),
                    skip_group_check=True)
            nc.vector.tensor_copy(out=g_sb[:, 7, 512:1024], in_=g7b)

            # sumx[c] = sum_l x[c, l] via PE against ones (same psum bank)
            sxt = g2ps.tile([128, 512], F32, name="g7")
            for cch in range(CT):
                for lt in range(NLT):
                    nc.tensor.matmul(
                        out=sxt[:, cch:cch + 1],
                        lhsT=xT[:, lt, cch * 128:(cch + 1) * 128],
                        rhs=ones_sb,
                        start=(cch == 0 and lt == 0), stop=(lt == NLT - 1),
                        skip_group_check=True)
            nc.vector.tensor_copy(out=sumx_sb, in_=sxt[:, 0:CT])

        # mirror: fill lower tiles (i>j) and upper col-7 from row 7
        with tc.tile_pool(name="mps", bufs=2, space="PSUM") as mps:
            def mirror(src_i, src_cols, dst_i, dst_cols):
                tp = mps.tile([128, 128], F16, name="mtp")
                nc.tensor.transpose(out=tp,
                                    in_=g_sb[:, src_i, src_cols],
                                    identity=idh_sb)
                pcopy(dst_i + dst_cols.start // 128,
                      g_sb[:, dst_i, dst_cols], tp)

            for i in range(7):
                for j in range(i + 1, 7):   # lower (j, i) from upper (i, j)
                    mirror(i, slice(j * 128, (j + 1) * 128),
                           j, slice(i * 128, (i + 1) * 128))
                # upper col 7 for row i from row-7 block i
                mirror(7, slice(i * 128, (i + 1) * 128),
                       i, slice(896, 1024))
        if dbg:
            for i in range(CT):
                nc.sync.dma_start(out=dbg["dbg_g"][i * 128:(i + 1) * 128, :],
                                  in_=g_sb[:, i, :])

        # =========== phase 1.5: stats -> affine; beta/u rows; scale wq/wk
        with tc.tile_pool(name="stat", bufs=1) as pst, \
             tc.tile_pool(name="stps", bufs=1, space="PSUM") as stps:
            # E[x^2]*L per channel = diag(G)
            dtmp = pst.tile([128, 128], F32, name="dtmp")
            for i in range(CT):
                nc.vector.tensor_mul(out=dtmp, in0=g_sb[:, i, i * 128:(i + 1) * 128],
                                     in1=idh_sb)
                nc.vector.tensor_reduce(out=ex2_sb[:, i:i + 1], in_=dtmp,
                                        axis=mybir.AxisListType.X, op=Alu.add)
            # tall: col 2i = mean, col 2i+1 = E[x^2]
            tall = pst.tile([128, 2 * CT], F32, name="tall")
            tr = tall.rearrange("p (t two) -> p t two", two=2)
            nc.vector.tensor_scalar_mul(out=tr[:, :, 0], in0=sumx_sb,
                                        scalar1=1.0 / L)
            nc.vector.tensor_scalar_mul(out=tr[:, :, 1], in0=ex2_sb,
                                        scalar1=1.0 / L)
            gst_ps = stps.tile([4, 2 * CT], F32, name="gst")
            nc.tensor.matmul(out=gst_ps, lhsT=gsel_sb, rhs=tall,
                             start=True, stop=True)
            gst_sb = pst.tile([4, 2 * CT], F32, name="gstsb")
            nc.vector.tensor_scalar_mul(out=gst_sb, in0=gst_ps, scalar1=1.0 / 32.0)
            chst_ps = stps.tile([128, 2 * CT], F32, name="chst")
            nc.tensor.matmul(out=chst_ps, lhsT=gbr_sb, rhs=gst_sb,
                             start=True, stop=True)
            ch = chst_ps.rearrange("p (t two) -> p t two", two=2)
            mu = pst.tile([128, CT], F32, name="mu")
            nc.vector.tensor_copy(out=mu, in_=ch[:, :, 0])
            var = pst.tile([128, CT], F32, name="var")
            nc.vector.tensor_mul(out=var, in0=mu, in1=mu)
            nc.vector.tensor_sub(out=var, in0=ch[:, :, 1], in1=var)
            nc.scalar.activation(out=var, in_=var, func=Act.Sqrt,
                                 bias=eps_sb, scale=1.0)
            nc.vector.reciprocal(out=var, in_=var)          # rstd
            nc.vector.tensor_mul(out=scale_sb, in0=var, in1=gnw_sb)
            nc.vector.tensor_mul(out=var, in0=mu, in1=scale_sb)
            nc.vector.tensor_sub(out=bias_sb, in0=gnb_sb, in1=var)
            nc.vector.tensor_copy(out=bcol16, in_=bias_sb)
            nc.vector.tensor_copy(out=bcolb, in_=bias_sb)
            nc.vector.tensor_copy(out=sumx16, in_=sumx_sb)

        with tc.tile_pool(name="rowps", bufs=2, space="PSUM") as rowps, \
             tc.tile_pool(name="pbrow", bufs=1) as pbrow:
            bqp_sb = pbrow.tile([1, C], F16, name="bqpr")
            nc.sync.dma_start(out=bqp_sb, in_=bqp[:, :])
            bqpL_sb = pbrow.tile([1, C], F16, name="bqpLr")
            nc.sync.dma_start(out=bqpL_sb, in_=bqpL[:, :])
            bkp_sb = pbrow.tile([1, C], F16, name="bkpr")
            nc.sync.dma_start(out=bkp_sb, in_=bkp[:, :])

            def matvec_row(col, wtile, out_row, post):
                for half in range(2):
                    hp = rowps.tile([1, 512], F32, name="rps")
                    for cch in range(CT):
                        nc.tensor.matmul(
                            out=hp, lhsT=col[:, cch:cch + 1],
                            rhs=wtile[:, cch, half * 512:(half + 1) * 512],
                            start=(cch == 0), stop=(cch == CT - 1))
                    post(out_row[:, half * 512:(half + 1) * 512], hp,
                         slice(half * 512, (half + 1) * 512))

            matvec_row(bcol16, wqt_sb, betaq_r,
                       lambda o, p, sl: (
                           nc.vector.tensor_add(out=o, in0=p, in1=bqp_sb[:, sl]),
                           nc.vector.scalar_tensor_tensor(
                               out=lbetaq_r[:, sl], in0=p, scalar=float(L),
                               in1=bqpL_sb[:, sl], op0=Alu.mult, op1=Alu.add)))
            matvec_row(bcol16, wkt_sb, betak_r,
                       lambda o, p, sl: nc.vector.tensor_add(
                           out=o, in0=p, in1=bkp_sb[:, sl]))
            # fold groupnorm scale into score weights (in place)
            for ct in range(CT):
                eng = nc.vector if ct % 2 == 0 else nc.gpsimd
                eng.tensor_scalar_mul(out=wqt_sb[:, ct, :], in0=wqt_sb[:, ct, :],
                                      scalar1=scale_sb[:, ct:ct + 1])
                eng2 = nc.gpsimd if ct % 2 == 0 else nc.vector
                eng2.tensor_scalar_mul(out=wkt_sb[:, ct, :], in0=wkt_sb[:, ct, :],
                                       scalar1=scale_sb[:, ct:ct + 1])
            matvec_row(sumx16, wqt_sb, uq_r,
                       lambda o, p, sl: nc.vector.tensor_copy(out=o, in_=p))
            matvec_row(sumx16, wkt_sb, uk_r,
                       lambda o, p, sl: nc.vector.tensor_copy(out=o, in_=p))

        # =========== phase 2: M2, score, softmax, A ======================
        pA = ctx.enter_context(tc.tile_pool(name="pA", bufs=1))
        a_sb = pA.tile([128, CT, C], BF16, name="asb")
        pwv = ctx.enter_context(tc.tile_pool(name="pwv", bufs=1))
        wvn_sb = pwv.tile([128, CT, C], BF16, name="wvn")
        wpt_sb = pwv.tile([128, CT, C], BF16, name="wpt")
        for ct in range(CT):
            nc.sync.dma_start(out=wvn_sb[:, ct, :],
                              in_=wvn[ct * 128:(ct + 1) * 128, :])
            nc.scalar.dma_start(out=wpt_sb[:, ct, :],
                                in_=wpt[ct * 128:(ct + 1) * 128, :])

        with tc.tile_pool(name="pm2", bufs=2) as pm2, \
             tc.tile_pool(name="m2ps", bufs=2, space="PSUM") as m2ps, \
             tc.tile_pool(name="scps", bufs=2, space="PSUM") as scps, \
             tc.tile_pool(name="aps", bufs=2, space="PSUM") as apsp:
            for kb in range(CT):        # head pair kb: heads 2kb, 2kb+1
                m2p = [m2ps.tile([128, 512], F32, name="m2p") for _ in range(2)]
                for cch in range(CT):
                    for dch in range(CT):
                        nc.tensor.matmul(
                            out=m2p[cch // 4][:, (cch % 4) * 128:
                                              (cch % 4) * 128 + 128],
                            lhsT=g_sb[:, dch, cch * 128:(cch + 1) * 128],
                            rhs=wkt_sb[:, dch, kb * 128:(kb + 1) * 128],
                            start=(dch == 0 and cch % 4 == 0),
                            stop=(dch == CT - 1),
                            skip_group_check=True)
                m2_sb = pm2.tile([128, CT, 128], F16, name="m2sb")
                for cch in range(CT):
                    pcopy(cch, m2_sb[:, cch, :],
                          m2p[cch // 4][:, (cch % 4) * 128:
                                        (cch % 4) * 128 + 128])
                sc = scps.tile([128, 128], F32, name="scp")
                for cch in range(CT):
                    nc.tensor.matmul(
                        out=sc, lhsT=wqt_sb[:, cch, kb * 128:(kb + 1) * 128],
                        rhs=m2_sb[:, cch, :],
                        start=(cch == 0), stop=False, skip_group_check=True)
                ksl = slice(kb * 128, (kb + 1) * 128)
                nc.tensor.matmul(out=sc, lhsT=betaq_r[:, ksl], rhs=uk_r[:, ksl],
                                 start=False, stop=False, skip_group_check=True)
                nc.tensor.matmul(out=sc, lhsT=uq_r[:, ksl], rhs=betak_r[:, ksl],
                                 start=False, stop=False, skip_group_check=True)
                nc.tensor.matmul(out=sc, lhsT=lbetaq_r[:, ksl], rhs=betak_r[:, ksl],
                                 start=False, stop=True, skip_group_check=True)
                # softmax for the two heads in this pair
                expt = prow.tile([128, 64], F32, name=f"expt{kb % 2}")
                for odd in range(2):
                    p0 = odd * 64
                    nc.vector.tensor_reduce(
                        out=negmax[p0:p0 + 64, kb:kb + 1],
                        in_=sc[p0:p0 + 64, p0:p0 + 64],
                        axis=mybir.AxisListType.X, op=Alu.max, negate=True)
                for odd in range(2):
                    p0 = odd * 64
                    nc.scalar.activation(
                        out=expt[p0:p0 + 64, :],
                        in_=sc[p0:p0 + 64, p0:p0 + 64], func=Act.Exp,
                        bias=negmax[p0:p0 + 64, kb:kb + 1], scale=1.0,
                        accum_out=sumexp[p0:p0 + 64, kb:kb + 1])
                nc.vector.reciprocal(out=rs[:, kb:kb + 1],
                                     in_=sumexp[:, kb:kb + 1])
                nc.vector.tensor_scalar_mul(
                    out=w_sb[:, kb * 64:(kb + 1) * 64], in0=expt,
                    scalar1=rs[:, kb:kb + 1])
                # A_pair = w_h^T Wp_h^T for both heads (disjoint partitions)
                aph = [apsp.tile([128, 512], F32, name="apt") for _ in range(2)]
                for odd in range(2):
                    p0 = odd * 64
                    for half in range(2):
                        nc.tensor.matmul(
                            out=aph[half][p0:p0 + 64, :],
                            lhsT=w_sb[p0:p0 + 64, kb * 64:(kb + 1) * 64],
                            rhs=wpt_sb[p0:p0 + 64, kb,
                                       half * 512:(half + 1) * 512],
                            start=True, stop=True, skip_group_check=True)
                for half in range(2):
                    pcopy(kb + half,
                          a_sb[:, kb, half * 512:(half + 1) * 512], aph[half])

        # =========== phase 2b: M^T build (+I), consts, fold affine =======
        pmt = ctx.enter_context(tc.tile_pool(name="pmt", bufs=1))
        mt_sb = pmt.tile([128, CT, C], BF16, name="mtsb")
        with tc.tile_pool(name="mtps", bufs=2, space="PSUM") as mtps, \
             tc.tile_pool(name="ccps", bufs=1, space="PSUM") as ccps:
            for cch in range(CT):
                for half in range(2):
                    mp = mtps.tile([128, 512], F32, name="mtp")
                    has_id = (cch // 4) == half
                    for kb in range(CT):
                        nc.tensor.matmul(
                            out=mp,
                            lhsT=wvn_sb[:, kb, cch * 128:(cch + 1) * 128],
                            rhs=a_sb[:, kb, half * 512:(half + 1) * 512],
                            start=(kb == 0),
                            stop=(kb == CT - 1 and not has_id),
                            skip_group_check=True)
                    if has_id:
                        off = (cch * 128) % 512
                        nc.tensor.matmul(
                            out=mp[:, off:off + 128], lhsT=idb_sb, rhs=idb_sb,
                            start=False, stop=True, skip_group_check=True)
                    pcopy(cch + half,
                          mt_sb[:, cch, half * 512:(half + 1) * 512], mp)
            # c0[o] = sum_pairs A_pair^T bv_pair ; c1 = M' b
            cc = ccps.tile([128, 2 * CT], F32, name="ccps")
            for och in range(CT):
                for kb in range(CT):
                    nc.tensor.matmul(
                        out=cc[:, och:och + 1],
                        lhsT=a_sb[:, kb, och * 128:(och + 1) * 128],
                        rhs=bvc_sb[:, kb:kb + 1],
                        start=(och == 0 and kb == 0), stop=(kb == CT - 1),
                        skip_group_check=True)
                for cch in range(CT):
                    nc.tensor.matmul(
                        out=cc[:, CT + och:CT + och + 1],
                        lhsT=mt_sb[:, cch, och * 128:(och + 1) * 128],
                        rhs=bcolb[:, cch:cch + 1],
                        start=False, stop=(cch == CT - 1),
                        skip_group_check=True)
            nc.vector.tensor_add(out=pbc0_sb, in0=cc[:, 0:CT], in1=pbc_sb)
            nc.vector.tensor_add(out=pbc0_sb, in0=pbc0_sb, in1=cc[:, CT:2 * CT])
        if dbg:
            nc.sync.dma_start(out=dbg["dbg_w"][:, :], in_=w_sb)
            for i in range(CT):
                nc.sync.dma_start(out=dbg["dbg_mt"][i * 128:(i + 1) * 128, :],
                                  in_=mt_sb[:, i, :])
            nc.sync.dma_start(out=dbg["dbg_pbc0"][:, :], in_=pbc0_sb)
            for i, r in enumerate([betaq_r, betak_r, lbetaq_r, uq_r, uk_r]):
                nc.sync.dma_start(out=dbg["dbg_rows"][i:i + 1, :], in_=r)
            nc.sync.dma_start(out=dbg["dbg_sc"][:, 0:CT], in_=sumx_sb)
            nc.sync.dma_start(out=dbg["dbg_sc"][:, CT:2 * CT], in_=scale_sb)

        # =========== phase 3: out = (diag(s) M'^T)^T xbf + pbc0 ==========
        with tc.tile_pool(name="outp", bufs=3) as poutp, \
             tc.tile_pool(name="fps", bufs=2, space="PSUM") as fps:
            # fold groupnorm scale into M'^T rows (in place)
            for ct in range(CT):
                eng = nc.vector if ct % 2 == 0 else nc.gpsimd
                eng.tensor_scalar_mul(out=mt_sb[:, ct, :], in0=mt_sb[:, ct, :],
                                      scalar1=scale_sb[:, ct:ct + 1])
            for lb in range(CT):        # 512-wide l blocks
                lsl = slice(lb * 512, (lb + 1) * 512)
                for ot in range(CT):
                    ps = fps.tile([128, 512], F32, name="fpst")
                    for cch in range(CT):
                        nc.tensor.matmul(
                            out=ps,
                            lhsT=mt_sb[:, cch, ot * 128:(ot + 1) * 128],
                            rhs=xbf[:, cch, lsl],
                            start=(cch == 0), stop=(cch == CT - 1))
                    ob = poutp.tile([128, 512], F32, name="outb")
                    if ot % 2 == 0:
                        nc.vector.tensor_scalar_add(out=ob, in0=ps,
                                                    scalar1=pbc0_sb[:, ot:ot + 1])
                    else:
                        nc.scalar.activation(out=ob, in_=ps, func=Act.Identity,
                                             bias=pbc0_sb[:, ot:ot + 1],
                                             scale=1.0)
                    qeng = nc.sync if ot % 2 == 0 else nc.scalar
                    qeng.dma_start(out=out[ot * 128:(ot + 1) * 128, lsl], in_=ob)


_NC_CACHE = {}


def _get_nc():
    if "nc" not in _NC_CACHE:
        _NC_CACHE["nc"] = _build()
    return _NC_CACHE["nc"]


def _host_prep(x, gn_w, gn_b, qkv_w, qkv_b, proj_w, proj_b):
    s = np.float32(1.0 / np.sqrt(np.sqrt(CH)))
    # reference splits qkv PER HEAD: channel block h*192..(h+1)*192 = [q_h|k_h|v_h]
    qw = qkv_w.reshape(H, 3, CH, C)
    qb3 = qkv_b.reshape(H, 3, CH)
    wq = np.ascontiguousarray(qw[:, 0].reshape(C, C)) * s   # head-major rows
    wk = np.ascontiguousarray(qw[:, 1].reshape(C, C)) * s
    wv = np.ascontiguousarray(qw[:, 2].reshape(C, C))
    bq = np.ascontiguousarray(qb3[:, 0].reshape(C)) * s
    bk = np.ascontiguousarray(qb3[:, 1].reshape(C)) * s
    bv = np.ascontiguousarray(qb3[:, 2].reshape(C))

    wqt_h = np.ascontiguousarray(wq.T).astype(np.float16)   # [c, q]
    wkt_h = np.ascontiguousarray(wk.T).astype(np.float16)
    wpt_h = np.ascontiguousarray(proj_w.T).astype(BFNP)     # [c_ctx, o]
    wvn_h = np.ascontiguousarray(wv).astype(BFNP)           # [vc, c]
    bqp_h = bq.reshape(1, C).astype(np.float16)
    bqpL_h = (bq * np.float32(L)).reshape(1, C).astype(np.float16)
    bkp_h = bk.reshape(1, C).astype(np.float16)
    bvc_h = np.ascontiguousarray(bv.reshape(CT, 128).T).astype(BFNP)
    pbc_h = np.ascontiguousarray(proj_b.reshape(CT, 128).T).astype(np.float32)
    gnw_h = np.ascontiguousarray(gn_w.reshape(CT, 128).T)
    gnb_h = np.ascontiguousarray(gn_b.reshape(CT, 128).T)
    gsel_h = np.zeros((128, 4), np.float32)
    for p in range(128):
        gsel_h[p, p // 32] = 1.0
    gbr_h = np.ascontiguousarray(gsel_h.T)
    idb_h = np.eye(128, dtype=np.float32).astype(BFNP)
    idh_h = np.eye(128, dtype=np.float32).astype(np.float16)
    ones_h = np.ones((128, 1), np.float32).astype(BFNP)
    base = {
        "wqt": wqt_h, "wkt": wkt_h, "wpt": wpt_h, "wvn": wvn_h,
        "bqp": bqp_h, "bqpL": bqpL_h, "bkp": bkp_h, "bvc": bvc_h,
        "pbc": pbc_h, "gnw": gnw_h, "gnb": gnb_h,
        "gsel": gsel_h, "gbr": gbr_h, "idb": idb_h, "idh": idh_h,
        "ones": ones_h,
    }
    in_maps = []
    for b in range(B):
        m = dict(base)
        m["x"] = np.ascontiguousarray(x[b])
        m["xtf"] = np.ascontiguousarray(x[b].T)   # pure layout permutation
        in_maps.append(m)
    return in_maps


def kernel(x, gn_w, gn_b, qkv_w, qkv_b, proj_w, proj_b):
    nc = _get_nc()
    in_maps = _host_prep(np.asarray(x, np.float32), np.asarray(gn_w, np.float32),
                         np.asarray(gn_b, np.float32), np.asarray(qkv_w, np.float32),
                         np.asarray(qkv_b, np.float32), np.asarray(proj_w, np.float32),
                         np.asarray(proj_b, np.float32))
    trace = bool(int(os.environ.get("ATT_TRACE", "0")))
    kwargs = {}
    if trace:
        kwargs = {"trace": True, "tmpdir": os.environ.get("ATT_TRACE_DIR", None)}
    res = run_bass_kernel_spmd(nc, in_maps, list(range(B)), **kwargs)
    out = np.stack([res.results[i]["out"] for i in range(B)], axis=0)
    if trace:
        kernel.last_exec_time_ns = res.exec_time_ns
    return out


kernel.last_exec_time_ns = None
